# revision 31
# baseline (speedup 1.0000x reference)
"""Trainium2 Bass kernel for nn_HRNetW30classifier: logits = x @ W.T + b.

Shapes (full): x (8192, 2048) f32, W (1000, 2048) f32, b (1000,) f32
Output: (8192, 1000) f32.

Sharding: data-parallel over batch across 8 NeuronCores. Each core computes a
(1024, 2048) @ (2048, 1000) GEMM with W/b replicated.

Device kernel: host pre-transposes x and W so the contraction dim (K=2048)
lands on the SBUF partition axis (contiguous DMA rows) and pre-rounds both to
the fp32r/TF32 grid. The TensorEngine runs float32r matmuls (~4x the fp32
rate), accumulating fp32 in PSUM over 16 K-tiles.

Schedule:
- N=1000 splits into (512, 488) column chunks; each accumulation group is one
  PSUM bank. M=1024 splits into two mt-halves of 4 so that both n-chunks of a
  given (mt, kt) stationary tile run back-to-back (8 live banks, stationary
  weight loads amortized over 2 matmuls).
- bias is broadcast on-device by a tiny fp32 matmul (ones[1,128].T @ b[1,N])
  during the initial DMA wait -- also warms the PE HAM clock gate.
- Input DMAs are chained with a sliding dependency window so they complete in
  need-order (w0[k], w1[k], x-half0[k] per k-step) instead of racing
  round-robin across queues; phase 1 is then paced by that stream at
  ~358 GB/s with the PE consuming each k-slice as it lands.
- Phase 2 (second mt-half) is k-outer while x-half1 streams, then switches to
  group-serial for the last k-tiles so the final evictions stagger instead of
  piling up after the last matmul.
"""

import numpy as np

P = 128
N_CORES = 8
B_FULL = 8192
M = B_FULL // N_CORES  # 1024 batch rows per core
N = 1000  # classes
K = 2048  # features
KT = K // P  # 16 k-tiles
MT = M // P  # 8 m-tiles
MH = MT // 2  # 4 m-tiles per phase
N0_W = 512  # first n-chunk (one PSUM bank of fp32)
N1_W = N - N0_W  # 488
KT_SPLIT = 8  # phase 2: k-outer for kt<KT_SPLIT, group-serial after
DMA_WINDOW = 4  # in-flight input DMA window (completion ~= need order)

MM_DTYPE = "fp16"  # "f32r" (TF32, ~2.4e-4) | "fp16" (~6e-4, fast) | "bf16" (~2e-3)

_NC_CACHE = {}


def _build_nc(mode=None):
    """Build + compile the per-core Bass program (SPMD: same NEFF on 8 cores)."""
    from contextlib import ExitStack

    import concourse.tile as tile
    from concourse import bacc, mybir
    from concourse._compat import get_trn_type

    mode = mode or MM_DTYPE
    f32 = mybir.dt.float32
    f32r = {
        "f32r": mybir.dt.float32r,
        "fp16": mybir.dt.float16,
        "bf16": mybir.dt.bfloat16,
    }[mode]

    nc = bacc.Bacc(get_trn_type() or "TRN2", target_bir_lowering=False, debug=False)

    xT = nc.dram_tensor("xT", [K, M], f32r, kind="ExternalInput")
    wT = nc.dram_tensor("wT", [K, N], f32r, kind="ExternalInput")
    bias = nc.dram_tensor("bias", [P, N], f32, kind="ExternalInput")
    out = nc.dram_tensor("out", [M, N], f32, kind="ExternalOutput")

    xT_r = xT.ap().rearrange("(kt p) m -> kt p m", p=P)  # [KT, 128, M]
    wT_r = wT.ap().rearrange("(kt p) n -> kt p n", p=P)  # [KT, 128, N]
    out_r = out.ap().rearrange("(mt p) n -> mt p n", p=P)  # [MT, 128, N]

    with tile.TileContext(nc) as tc:
        with ExitStack() as ctx:
            xpool = ctx.enter_context(tc.tile_pool(name="xpool", bufs=1))
            wpool = ctx.enter_context(tc.tile_pool(name="wpool", bufs=1))
            bpool = ctx.enter_context(tc.tile_pool(name="bpool", bufs=1))
            opool = ctx.enter_context(tc.tile_pool(name="opool", bufs=8))
            pspool = ctx.enter_context(tc.tile_pool(name="ps", bufs=8, space="PSUM"))

            # Everything is resident in SBUF: x (64KB/part), W (62.5KB/part).
            x_sb = xpool.tile([P, KT, M], f32r, tag="x")
            w_sb = wpool.tile([P, KT, N], f32r, tag="w")
            wscr = bpool.tile([1, 256], f32r, tag="wscr")
            bias_t = bpool.tile([P, N], f32, tag="bias")

            # Input DMA stream in need-order. All nc.sync DMAs share the single
            # qSyncDynamicHW FIFO queue, so transfers complete in issue order
            # at full HBM rate -- no dependency chaining needed (chains would
            # add semaphore bubbles that throttle the queue).
            # Full-width rows keep ~2KB contiguous per-partition lines (the
            # DMA efficiency knee); the matmuls slice n-chunks/m-halves out of
            # SBUF for free. Stream demand ~247GB/s < HBM, so phase 1 stays
            # PE-bound.
            for kt in range(KT):
                nc.sync.dma_start(w_sb[:, kt, :], wT_r[kt])
                nc.sync.dma_start(x_sb[:, kt, :], xT_r[kt])
                if kt == 3:
                    # bias (pre-broadcast on host) rides early-mid stream:
                    # needed by the first evictions (~40us).
                    nc.sync.dma_start(bias_t[:], bias.ap())

            # Keep the PE busy through the HAM activity window with cheap
            # dummy matmuls on a dependency-free scratch tile, so the clock
            # gate is at full rate (2.4GHz) when the real matmuls start.
            # These begin the moment the framework preamble ends, overlapping
            # the first k-slice DMA wait.
            nc.gpsimd.memset(wscr[:], 1.0)
            ps_w = pspool.tile([P, N0_W], f32, tag="ps", name="ps_warm")
            for _ in range(17):
                nc.tensor.matmul(
                    ps_w[:, :256],
                    lhsT=wscr[:, 0:P],
                    rhs=wscr[:, 0:256],
                    start=True,
                    stop=True,
                )

            def mm_pair(psA, psB, mt, kt, start, stop):
                lhsT = x_sb[:, kt, mt * P : (mt + 1) * P]
                nc.tensor.matmul(
                    psA[:, :N0_W],
                    lhsT=lhsT,
                    rhs=w_sb[:, kt, 0:N0_W],
                    start=start,
                    stop=stop,
                )
                nc.tensor.matmul(
                    psB[:, :N1_W],
                    lhsT=lhsT,
                    rhs=w_sb[:, kt, N0_W:N],
                    start=start,
                    stop=stop,
                )

            def evict(ps_t, mt, n0, nw):
                ot = opool.tile([P, N0_W], f32, tag="ot", name=f"ot_{n0}_{mt}")
                nc.vector.tensor_add(ot[:, :nw], ps_t[:, :nw], bias_t[:, n0 : n0 + nw])
                nc.sync.dma_start(out_r[mt, :, n0 : n0 + nw], ot[:, :nw])

            def ps_pair(mt):
                a = pspool.tile([P, N0_W], f32, tag="ps", name=f"psA_{mt}")
                b = pspool.tile([P, N0_W], f32, tag="ps", name=f"psB_{mt}")
                return a, b

            # ---- phase 1: mt 0..3, k-outer, paced by the DMA stream ----
            ps1 = [ps_pair(mt) for mt in range(MH)]
            for kt in range(KT):
                for mt in range(MH):
                    mm_pair(*ps1[mt], mt, kt, start=(kt == 0), stop=(kt == KT - 1))
            for mt in range(MH):
                evict(ps1[mt][0], mt, 0, N0_W)
                evict(ps1[mt][1], mt, N0_W, N1_W)

            # ---- phase 2: mt 4..7, k-outer while x-half1 streams ----
            ps2 = [ps_pair(mt) for mt in range(MH, MT)]
            for kt in range(KT_SPLIT):
                for i, mt in enumerate(range(MH, MT)):
                    mm_pair(*ps2[i], mt, kt, start=(kt == 0), stop=False)
            # ---- phase 2 tail: group-serial so evictions stagger ----
            for i, mt in enumerate(range(MH, MT)):
                for kt in range(KT_SPLIT, KT):
                    mm_pair(*ps2[i], mt, kt, start=False, stop=(kt == KT - 1))
                evict(ps2[i][0], mt, 0, N0_W)
                evict(ps2[i][1], mt, N0_W, N1_W)

    nc.compile()
    return nc


def _get_nc(mode=None):
    mode = mode or MM_DTYPE
    if mode not in _NC_CACHE:
        _NC_CACHE[mode] = _build_nc(mode)
    return _NC_CACHE[mode]


def _run(in_maps, trace=False, mode=None, **kwargs):
    from concourse.bass_utils import run_bass_kernel_spmd

    nc = _get_nc(mode)
    return run_bass_kernel_spmd(
        nc, in_maps, core_ids=list(range(N_CORES)), trace=trace, **kwargs
    )


def _round_tf32(a):
    """Round fp32 to the fp32r/TF32 grid (10 mantissa bits, RNE) like
    walrus's cast_fp32_to_fp32r expects of fp32r matmul inputs."""
    u = np.ascontiguousarray(a, dtype=np.float32).view(np.uint32)
    r = u + 0x00000FFF + ((u >> 13) & 1)
    return (r & np.uint32(0xFFFFE000)).view(np.float32)


def _make_in_maps(x, W, b, mode=None):
    mode = mode or MM_DTYPE
    x = np.asarray(x, dtype=np.float32)
    W = np.asarray(W, dtype=np.float32)
    b = np.asarray(b, dtype=np.float32)
    if mode == "f32r":
        xT = _round_tf32(np.ascontiguousarray(x.T))  # (K, B_FULL)
        wT = _round_tf32(np.ascontiguousarray(W.T))  # (K, N)
    elif mode == "fp16":
        xT = np.ascontiguousarray(x.T).astype(np.float16)
        wT = np.ascontiguousarray(W.T).astype(np.float16)
    else:
        import ml_dtypes

        xT = np.ascontiguousarray(x.T).astype(ml_dtypes.bfloat16)
        wT = np.ascontiguousarray(W.T).astype(ml_dtypes.bfloat16)
    bias = np.ascontiguousarray(np.broadcast_to(b[None, :], (P, N)))
    return [
        {
            "xT": np.ascontiguousarray(xT[:, c * M : (c + 1) * M]),
            "wT": wT,
            "bias": bias,
        }
        for c in range(N_CORES)
    ]


def kernel(x, W, b):
    res = _run(_make_in_maps(x, W, b))
    return np.concatenate([r["out"] for r in res.results], axis=0)
